# revision 38
# baseline (speedup 1.0000x reference)
"""Trainium2 Bass kernel for nn_MultiHeadedAttentionHighDim.

Reference computation (B=4, S=64, T=128, D=512, H=8, DK=64):
  main branch : per (b, s): multi-head attention over time
                q/k/v = x @ W.T + b ; scores = q kT / sqrt(DK); softmax; out proj Wo
  sensor branch: qs = (q @ Wqs.T + bqs).reshape(B,S,T*D); same for ks;
                scores_s = qs ksT / sqrt(T*D); beta-softmax -> attn_s [B,1,S,S]

Sharding over 8 cores: core c = (b, half) with b = c//2, half = c%2.
  - main branch: data parallel over sensors (32 sensors per core).
  - sensor branch: G[b,i,j] = sum_t (q[b,i,t,:] @ M) . k[b,j,t,:] with
    M = Wqs.T @ Wks  (precomputed on host) -- sharded over the time axis
    (64 t-steps per core); host sums the two partial G's per batch and
    applies the tiny beta-softmax on the CPU (16K elements).

Device layouts (per core, all bf16 unless noted):
  xqT/xkT/xvT [D=512, ROWS=4096]   transposed main-branch inputs (rows=(s,t))
  qsen [512, 4096]                  sensor q, layout [e, (t,i)]
  ksen [128, 16384]                 sensor k, layout [dp, (t, dc, j)]
  wqT/wkT/wvT/woT/msb [512, 512]    weights (pre-transposed as needed)
  bq_t/bk_t [128, 4] f32            per-partition bias per d-chunk
Outputs: out [4096, 512] f32, g [64, 64] f32 (partial Gram).

Hardware constraints honored here (each found the hard way):
  - every DMA-written SBUF region is written exactly once (raw scoped sbuf
    tensors, no pool-slot reuse): a reused slot makes the next DMA carry two
    sync waits, and the DMA descriptor only has one wait field.
  - NO matmul operands at partition base 64: mixing base-0 and base-64 K=64
    matmuls in one kernel aborts NEFF execution on hardware. All matmuls here
    contract over the full 128 partitions at base 0; per-head separation is
    done with zero-masked copies of K^T (ktz) and zero-interleaved V (v_bz).
  - main-phase raw SBUF tensors reuse the sensor phase's freed address range,
    which Tile's per-tensor dep tracking cannot see -> explicit add_dep edges.
"""

import sys

if "/opt/trn_rl_repo" not in sys.path:
    sys.path.insert(0, "/opt/trn_rl_repo")

import numpy as np
import ml_dtypes

BF16 = ml_dtypes.bfloat16

B, S, T, D = 4, 64, 128, 512
H, DK = 8, 64
NCORES = 8
SH = S // 2          # sensors per core (main branch)
ROWS = SH * T        # 4096 rows per core
TH = T // 2          # time steps per core (sensor branch)
BLK = 4              # sensors per main-phase block
RB = BLK * T         # 512 rows per block
NBLK = ROWS // RB    # 8 blocks

_CACHE: dict = {}
VPROJ = ('vproj', 1)
BUFS = {'blk': 2, 'sm': 4, 'pp': 2, 'sc': 1, 'dn': 1, 'pv': 1}


def _build_program(has_bv: bool, has_bo: bool):
    import concourse.bacc as bacc
    import concourse.bass as bass
    import concourse.mybir as mybir
    import concourse.tile as tile
    import bass_rust as _bass_rust
    from contextlib import ExitStack

    f32 = mybir.dt.float32
    bf16 = mybir.dt.bfloat16
    AF = mybir.ActivationFunctionType
    ALU = mybir.AluOpType

    nc = bacc.Bacc(trn_type="TRN2", target_bir_lowering=False, num_devices=NCORES)

    xqT = nc.dram_tensor("xqT", [D, ROWS], bf16, kind="ExternalInput")
    xkT = nc.dram_tensor("xkT", [D, ROWS], bf16, kind="ExternalInput")
    xvT = nc.dram_tensor("xvT", [D, ROWS], bf16, kind="ExternalInput")
    qsen = nc.dram_tensor("qsen", [D, S * TH], bf16, kind="ExternalInput")
    ksen = nc.dram_tensor("ksen", [128, TH * 4 * S], bf16, kind="ExternalInput")
    wqT = nc.dram_tensor("wqT", [D, D], bf16, kind="ExternalInput")
    wkT = nc.dram_tensor("wkT", [D, D], bf16, kind="ExternalInput")
    wvT = nc.dram_tensor("wvT", [D, D], bf16, kind="ExternalInput")
    woT = nc.dram_tensor("woT", [D, D], bf16, kind="ExternalInput")
    msb = nc.dram_tensor("msb", [D, D], bf16, kind="ExternalInput")
    bq_t = nc.dram_tensor("bq_t", [128, 4], f32, kind="ExternalInput")
    bk_t = nc.dram_tensor("bk_t", [128, 4], f32, kind="ExternalInput")
    bv_r = nc.dram_tensor("bv_r", [1, D], bf16, kind="ExternalInput")
    bo_r = nc.dram_tensor("bo_r", [1, D], bf16, kind="ExternalInput")

    out_d = nc.dram_tensor("out", [ROWS, D], bf16, kind="ExternalOutput")
    g_d = nc.dram_tensor("g", [S, S], f32, kind="ExternalOutput")

    with tile.TileContext(nc) as tc, ExitStack() as ctx:
        consts = ctx.enter_context(tc.tile_pool(name="consts", bufs=1))
        blk = ctx.enter_context(tc.tile_pool(name="blk", bufs=BUFS["blk"]))
        sm = ctx.enter_context(tc.tile_pool(name="sm", bufs=BUFS["sm"]))
        pp = ctx.enter_context(tc.tile_pool(name="pp", bufs=BUFS["pp"], space="PSUM"))
        ps_sc = ctx.enter_context(tc.tile_pool(name="ps_sc", bufs=BUFS["sc"], space="PSUM"))
        ps_dn = ctx.enter_context(tc.tile_pool(name="ps_dn", bufs=BUFS["dn"], space="PSUM"))
        ps_pv = ctx.enter_context(tc.tile_pool(name="ps_pv", bufs=BUFS["pv"], space="PSUM"))

        def load_w(dram, tag):
            t = consts.tile([128, 4, D], bf16, tag=tag, name=tag)
            nc.sync.dma_start(t, dram[:, :].rearrange("(ec ep) d -> ep ec d", ep=128))
            return t

        wq_sb = load_w(wqT, "wq_sb")
        wk_sb = load_w(wkT, "wk_sb")
        wv_sb = load_w(wvT, "wv_sb")
        wo_sb = load_w(woT, "wo_sb")
        m_sb = load_w(msb, "m_sb")
        bq_sb = consts.tile([128, 4], f32)
        nc.sync.dma_start(bq_sb, bq_t[:, :])
        bk_sb = consts.tile([128, 4], f32)
        nc.sync.dma_start(bk_sb, bk_t[:, :])
        ones128 = consts.tile([128, 128], bf16)
        nc.vector.memset(ones128, 1.0)
        if has_bv or has_bo:
            ones_col = consts.tile([1, 128], bf16)
            nc.vector.memset(ones_col, 1.0)
            bv_sb = consts.tile([1, D], bf16)
            nc.sync.dma_start(bv_sb, bv_r[:, :])
            bo_sb = consts.tile([1, D], bf16)
            nc.sync.dma_start(bo_sb, bo_r[:, :])

        # ---------------- sensor branch (time-sharded Gram) ----------------
        # qtil[(d), (t,i)] = sum_e M[e,d] * qsen[e,(t,i)]   (q~ = q @ M, transposed)
        # g[i, j] += sum_{t,dp} qtil[dp@dc, (t,i)] * ksen[dp, (t,dc,j)]
        qsen_v = qsen[:, :].rearrange("(ec ep) ti -> ep ec ti", ep=128)
        g_acc = sm.tile([64, 64], f32, tag="gacc")
        with (
            nc.sbuf_tensor([128, 4, S * TH], bf16) as qs_sb,
            nc.sbuf_tensor([128, TH * 4 * S], bf16) as ks_sb,
        ):
            for tt in range(8):
                nc.sync.dma_start(
                    qs_sb[:, :, tt * 512:(tt + 1) * 512],
                    qsen_v[:, :, tt * 512:(tt + 1) * 512])
                nc.sync.dma_start(
                    ks_sb[:, tt * 2048:(tt + 1) * 2048],
                    ksen[:, tt * 2048:(tt + 1) * 2048])
            for tt in range(8):  # 8 tiles of 512 (t,i) pairs = 8 t-values each
                tsl = slice(tt * 512, (tt + 1) * 512)
                qt_t = blk.tile([128, 4, 512], bf16, tag="qtil")
                for dc in range(4):
                    ps = pp.tile([128, 512], f32, tag="proj")
                    for ec in range(4):
                        nc.tensor.matmul(
                            ps, m_sb[:, ec, dc * 128:(dc + 1) * 128], qs_sb[:, ec, tsl],
                            start=(ec == 0), stop=(ec == 3))
                    nc.scalar.copy(qt_t[:, dc, :], ps)
                g_ps_full = ps_pv.tile([128, 512], f32, tag="pv", name="g_ps")
                g_ps = g_ps_full[:64, :64]
                n_mm = 0
                for t8 in range(8):
                    for dc in range(4):
                        c0 = (tt * 2048) + (t8 * 4 + dc) * 64
                        nc.tensor.matmul(
                            g_ps,
                            qt_t[:, dc, t8 * 64:(t8 + 1) * 64],
                            ks_sb[:, c0:c0 + 64],
                            start=(n_mm == 0), stop=(n_mm == 31))
                        n_mm += 1
                if tt == 0:
                    last_sensor = nc.vector.tensor_copy(g_acc, g_ps)
                else:
                    last_sensor = nc.vector.tensor_tensor(g_acc, g_acc, g_ps, ALU.add)
            nc.sync.dma_start(g_d[:, :], g_acc)

        # ---------------- main branch (per 4-sensor block) ----------------
        xq_v = xqT[:, :].rearrange("(ec ep) r -> ep ec r", ep=128)
        xk_v = xkT[:, :].rearrange("(ec ep) r -> ep ec r", ep=128)
        xv_v = xvT[:, :].rearrange("(ec ep) r -> ep ec r", ep=128)
        with (
            nc.sbuf_tensor([128, 4, ROWS], bf16) as xq_sb,
            nc.sbuf_tensor([128, 4, ROWS], bf16) as xk_sb,
            nc.sbuf_tensor([128, 4, ROWS], bf16) as xv_sb,
        ):
            # xq/xk/xv reuse the SBUF range just freed by qs_sb/ks_sb, which
            # Tile's per-tensor dep tracking cannot see -- every main input
            # DMA must wait for the end of the sensor phase explicitly.
            for rb in range(NBLK):
                rsl = slice(rb * RB, (rb + 1) * RB)
                for dst_sb, src_v in ((xq_sb, xq_v), (xk_sb, xk_v), (xv_sb, xv_v)):
                    dma = nc.sync.dma_start(dst_sb[:, :, rsl], src_v[:, :, rsl])
                    _bass_rust.add_dep_helper(
                        dma.ins, last_sensor.ins, sync=True,
                        reason="main input DMA reuses sensor-phase SBUF range")

            for rb in range(NBLK):
                rsl = slice(rb * RB, (rb + 1) * RB)
                # Q^T projection: [d-chunk, rows] with per-partition bias
                qt_b = blk.tile([128, 4, RB], bf16, tag="qtb")
                for dc in range(4):
                    ps = pp.tile([128, RB], f32, tag="proj")
                    for ec in range(4):
                        nc.tensor.matmul(
                            ps, wq_sb[:, ec, dc * 128:(dc + 1) * 128],
                            xq_sb[:, ec, rsl],
                            start=(ec == 0), stop=(ec == 3))
                    nc.scalar.activation(
                        qt_b[:, dc, :], ps, AF.Identity,
                        bias=bq_sb[:, dc:dc + 1], scale=1.0)
                # K^T projection into zero-masked per-parity copies:
                #   ktz[:, 2*dc+0] = [K^T(head 2dc) on parts 0:64 | zeros]
                #   ktz[:, 2*dc+1] = [zeros | K^T(head 2dc+1) on parts 64:128]
                ktz = blk.tile([128, 8, RB], bf16, tag="ktz")
                ktz_v = ktz.rearrange("p (dc pr) r -> p pr dc r", pr=2)
                nc.gpsimd.memset(ktz_v[64:128, 0], 0.0)
                nc.gpsimd.memset(ktz_v[0:64, 1], 0.0)
                for dc in range(4):
                    ps = pp.tile([128, RB], f32, tag="proj")
                    for ec in range(4):
                        nc.tensor.matmul(
                            ps, wk_sb[:, ec, dc * 128:(dc + 1) * 128],
                            xk_sb[:, ec, rsl],
                            start=(ec == 0), stop=(ec == 3))
                    nc.scalar.activation(
                        ktz[0:64, 2 * dc, :], ps[0:64, :], AF.Identity,
                        bias=bk_sb[0:64, dc:dc + 1], scale=1.0)
                    nc.scalar.activation(
                        ktz[64:128, 2 * dc + 1, :], ps[64:128, :], AF.Identity,
                        bias=bk_sb[64:128, dc:dc + 1], scale=1.0)
                # V projection into zero-interleaved layout per row-chunk:
                #   v_bz[:, rc, (2c+0)*128 :] = [V(head 2c) | 0]
                #   v_bz[:, rc, (2c+1)*128 :] = [0 | V(head 2c+1)]
                v_bz = blk.tile([128, 4, 2 * D], bf16, tag="vbz")
                vtag, vbufs = VPROJ
                v_bz_v = v_bz.rearrange("p rc (c x) -> p rc c x", c=4)
                nc.gpsimd.memset(v_bz_v[:, :, :, 64:192], 0.0)
                for rc in range(4):
                    ps = pp.tile([128, D], f32, tag=vtag, bufs=vbufs, name="psv")
                    for ec in range(4):
                        nc.tensor.matmul(
                            ps, xv_sb[:, ec, rb * RB + rc * 128:rb * RB + (rc + 1) * 128],
                            wv_sb[:, ec, :],
                            start=(ec == 0), stop=(ec == 3) and not has_bv)
                    if has_bv:
                        nc.tensor.matmul(ps, ones_col, bv_sb, start=False, stop=True)
                    # dest cols: head (c, parity), dk -> c*256 + parity*192 + dk
                    base = v_bz[:, rc, :]
                    dst = bass.AP(
                        tensor=base.tensor, offset=base.offset,
                        ap=[list(base.ap[0]), [256, 4], [192, 2], [1, 64]])
                    nc.vector.tensor_copy(
                        dst, ps[:, :].rearrange("p (c pr k) -> p c pr k", c=4, pr=2))

                # attention per sensor
                xt_b = blk.tile([128, 4, RB], bf16, tag="xtb")
                for sl in range(BLK):
                    csl = slice(sl * 128, (sl + 1) * 128)
                    pt = sm.tile([128, 1024], bf16, tag="pt")
                    for hf in range(2):
                        hsl = slice(hf * 512, (hf + 1) * 512)
                        s_sc = ps_sc.tile([128, 512], f32, tag="sc", name="s_sc")
                        for hh in range(4):
                            h = hf * 4 + hh
                            dc = h // 2
                            nc.tensor.matmul(
                                s_sc[:, hh * 128:(hh + 1) * 128],
                                ktz[:, 2 * dc + (h % 2), csl],
                                qt_b[:, dc, csl],
                                start=True, stop=True)
                        expm = sm.tile([128, 512], bf16, tag="expm", name="expm")
                        nc.scalar.activation(expm, s_sc, AF.Exp, scale=0.125)
                        dn = ps_dn.tile([128, 512], f32, tag="dn", name="dn")
                        nc.tensor.matmul(dn, ones128, expm,
                                         start=True, stop=True)
                        rdn = sm.tile([128, 512], f32, tag="rdn", name="rdn")
                        nc.vector.reciprocal_approx_fast(out=rdn, in_=dn)
                        nc.vector.tensor_tensor(
                            pt[:, hsl], expm, rdn, ALU.mult)
                    # PV: per chunk c, two accumulating full-partition matmuls
                    #   [numT(2c); numT(2c+1)] = [V_2c|0]^T P_2c + [0|V_2c+1]^T P_2c+1
                    pv = ps_pv.tile([128, 512], f32, tag="pv")
                    for c in range(4):
                        nc.tensor.matmul(
                            pv[:, c * 128:(c + 1) * 128],
                            v_bz[:, sl, (2 * c) * 128:(2 * c + 1) * 128],
                            pt[:, (2 * c) * 128:(2 * c + 1) * 128],
                            start=True, stop=False)
                        nc.tensor.matmul(
                            pv[:, c * 128:(c + 1) * 128],
                            v_bz[:, sl, (2 * c + 1) * 128:(2 * c + 2) * 128],
                            pt[:, (2 * c + 1) * 128:(2 * c + 2) * 128],
                            start=False, stop=True)
                    nc.scalar.copy(
                        xt_b[:, :, csl],
                        pv[:, :].rearrange("p (dc t) -> p dc t", dc=4))

                # output projection per sensor
                for sl in range(BLK):
                    csl = slice(sl * 128, (sl + 1) * 128)
                    ps = pp.tile([128, D], f32, tag="oproj", bufs=2)
                    for dc in range(4):
                        nc.tensor.matmul(
                            ps, xt_b[:, dc, csl], wo_sb[:, dc, :],
                            start=(dc == 0), stop=(dc == 3) and not has_bo)
                    if has_bo:
                        nc.tensor.matmul(ps, ones_col, bo_sb, start=False, stop=True)
                    ob = sm.tile([128, D], bf16, tag="ob")
                    if sl % 2 == 0:
                        nc.vector.tensor_copy(ob, ps)
                    else:
                        nc.scalar.copy(ob, ps)
                    nc.sync.dma_start(
                        out_d[(rb * 4 + sl) * 128:(rb * 4 + sl + 1) * 128, :], ob)

    nc.compile()
    return nc


def _get_program(has_bv: bool, has_bo: bool):
    key = (has_bv, has_bo, tuple(sorted(BUFS.items())))
    if key not in _CACHE:
        _CACHE[key] = _build_program(has_bv, has_bo)
    return _CACHE[key]


def _numpy_reference(query, key, value, mask, beta,
                     Wq, bq, Wk, bk, Wv, bv, Wo, bo,
                     Wqs, bqs, Wks, bks, Wvs, bvs):
    """Slow but exact fallback (used only for inputs the fast path doesn't
    cover, e.g. a mask with zeros -- the benchmark always passes all-ones)."""
    b, s, t, d = query.shape
    qs = (query.reshape(-1, d) @ Wqs.T + bqs).reshape(b, s, t * d)
    kss = (key.reshape(-1, d) @ Wks.T + bks).reshape(b, s, t * d)
    scores_s = np.einsum("bsd,btd->bst", qs, kss) / np.sqrt(np.float32(t * d))
    m = scores_s.max(axis=-1, keepdims=True)
    num = np.exp((scores_s - m) * beta)
    attn_s = (num / num.sum(axis=-1, keepdims=True))[:, None]

    def proj(x, W, bias):
        return ((x.reshape(-1, d) @ W.T + bias)
                .reshape(b, s, t, H, DK).transpose(0, 1, 3, 2, 4))

    q = proj(query, Wq, bq)
    k = proj(key, Wk, bk)
    v = proj(value, Wv, bv)
    scores = np.einsum("bshtd,bshud->bshtu", q, k) / np.sqrt(np.float32(DK))
    scores = np.where(mask[:, :, None], scores, np.float32(-1e9))
    mm = scores.max(axis=-1, keepdims=True)
    p = np.exp(scores - mm)
    p = p / p.sum(axis=-1, keepdims=True)
    x = np.einsum("bshtu,bshud->bshtd", p, v)
    x = x.transpose(0, 1, 3, 2, 4).reshape(b, s, t, d)
    out = x.reshape(-1, d) @ Wo.T + bo
    return out.reshape(b, s, t, d).astype(np.float32), attn_s.astype(np.float32)


def _make_in_maps(query, key, value, Ws):
    qb = query.astype(BF16)
    kb = key.astype(BF16)
    vb = value.astype(BF16)
    M = (Ws["Wqs"].T @ Ws["Wks"]).astype(BF16)

    shared = {
        "wqT": np.ascontiguousarray(Ws["Wq"].astype(BF16).T),
        "wkT": np.ascontiguousarray(Ws["Wk"].astype(BF16).T),
        "wvT": np.ascontiguousarray(Ws["Wv"].astype(BF16).T),
        "woT": np.ascontiguousarray(Ws["Wo"].astype(BF16).T),
        "msb": np.ascontiguousarray(M),
        "bq_t": np.ascontiguousarray(Ws["bq"].reshape(4, 128).T),
        "bk_t": np.ascontiguousarray(Ws["bk"].reshape(4, 128).T),
        "bv_r": Ws["bv"].astype(BF16).reshape(1, D),
        "bo_r": Ws["bo"].astype(BF16).reshape(1, D),
    }

    in_maps = []
    for c in range(NCORES):
        b, half = c // 2, c % 2
        s0 = half * SH
        th0 = half * TH
        m = dict(shared)
        m["xqT"] = np.ascontiguousarray(qb[b, s0:s0 + SH].reshape(ROWS, D).T)
        m["xkT"] = np.ascontiguousarray(kb[b, s0:s0 + SH].reshape(ROWS, D).T)
        m["xvT"] = np.ascontiguousarray(vb[b, s0:s0 + SH].reshape(ROWS, D).T)
        # qsen [e, (t, i)] from [i, t, e]
        m["qsen"] = np.ascontiguousarray(
            qb[b, :, th0:th0 + TH, :].transpose(2, 1, 0).reshape(D, TH * S))
        # ksen [dp, (t, dc, j)] from [j, t, e=(dc,dp)]
        m["ksen"] = np.ascontiguousarray(
            kb[b, :, th0:th0 + TH, :].reshape(S, TH, 4, 128)
            .transpose(3, 1, 2, 0).reshape(128, TH * 4 * S))
        in_maps.append(m)
    return in_maps


def kernel(query, key, value, mask, beta,
           Wq, bq, Wk, bk, Wv, bv, Wo, bo,
           Wqs, bqs, Wks, bks, Wvs, bvs):
    from concourse.bass_utils import run_bass_kernel_spmd

    query = np.asarray(query, dtype=np.float32)
    key = np.asarray(key, dtype=np.float32)
    value = np.asarray(value, dtype=np.float32)
    mask = np.asarray(mask)
    beta = np.asarray(beta, dtype=np.float32)
    Ws = {n: np.asarray(w, dtype=np.float32) for n, w in (
        ("Wq", Wq), ("bq", bq), ("Wk", Wk), ("bk", bk), ("Wv", Wv), ("bv", bv),
        ("Wo", Wo), ("bo", bo), ("Wqs", Wqs), ("bqs", bqs), ("Wks", Wks),
        ("bks", bks))}

    if not mask.all():
        return _numpy_reference(query, key, value, mask, beta,
                                Ws["Wq"], Ws["bq"], Ws["Wk"], Ws["bk"],
                                Ws["Wv"], Ws["bv"], Ws["Wo"], Ws["bo"],
                                Ws["Wqs"], Ws["bqs"], Ws["Wks"], Ws["bks"],
                                Wvs, bvs)

    has_bv = bool(np.any(Ws["bv"]))
    has_bo = bool(np.any(Ws["bo"]))
    nc = _get_program(has_bv, has_bo)
    in_maps = _make_in_maps(query, key, value, Ws)

    res = run_bass_kernel_spmd(nc, in_maps, list(range(NCORES))).results

    out = np.empty((B, S, T, D), dtype=np.float32)
    G = np.zeros((B, S, S), dtype=np.float32)
    for c in range(NCORES):
        b, half = c // 2, c % 2
        s0 = half * SH
        out[b, s0:s0 + SH] = res[c]["out"].astype(np.float32).reshape(SH, T, D)
        G[b] += res[c]["g"]

    # bias corrections for the sensor branch (zero biases -> skip)
    if np.any(Ws["bqs"]) or np.any(Ws["bks"]):
        Qsum = query.sum(axis=2)  # [B, S, D]
        Ksum = key.sum(axis=2)
        u = Ws["bks"] @ Ws["Wqs"]  # [D]
        v = Ws["bqs"] @ Ws["Wks"]
        G += (Qsum @ u)[:, :, None]
        G += (Ksum @ v)[:, None, :]
        G += np.float32(T) * np.float32(Ws["bqs"] @ Ws["bks"])

    scores_s = G / np.sqrt(np.float32(T * D))
    mx = scores_s.max(axis=-1, keepdims=True)
    num = np.exp((scores_s - mx) * beta)
    attn_s = (num / num.sum(axis=-1, keepdims=True))[:, None].astype(np.float32)

    return out, attn_s
